# revision 28
# baseline (speedup 1.0000x reference)
"""Trainium2 Bass kernel for cross-attention (b=4, nq=2048, nkv=1024,
qdim=1024, cdim=768, heads=16, dim_head=64).

Sharding: 8 cores = batch(4) x nq-half(2). Each core computes a disjoint
[1024, 1024] slice of the output; no collectives needed.

Key compaction: the boolean key-mask drops ~half the keys, and masked keys
contribute exactly zero to softmax(QK^T)V, so the host gathers each batch's
unmasked context rows and pads to NKV2=640 (seed-0 batches have <=536 live
keys; 640 = mean + 8 sigma of Binomial(1024, 1/2), and padding rows are
forced to zero attention via the exp bias). All key-side work (K/V
projections, S, exp, O) shrinks by 640/1024.

Per-core algorithm (bf16 operands, f32 PSUM accumulation):
  CT = ctx_packed^T, XT = x^T       (xbar DMA-transposed loads, no PE time)
  KT = Wk^T @ CT   [inner, nkv2]    (inner chunk p holds heads 2p, 2p+1)
  QT = Wq^T @ XT   [inner, nq]
  V2 = CT^T @ Wv   [nkv2, inner]    stored as per-(chunk, head) blocks
                                    [V_h (64 cols) | ones (64 cols)]
  per hf (nq 512-half), per head-pair p, per key-chunk c:
    S^T = K_h @ Q_h^T               (row-tiled K=64: 2 heads concurrent)
    ES  = exp(SCALE*S^T + mbias_j)  (ScalarE; mbias = 0 live / -1e30 pad)
    po_h += [V_h | ones]^T @ ES_h   (M=128: rows 0-63 = O^T, 64-127 = the
                                     softmax denominator, broadcast x64)
  ot_h = po_h[0:64] * 1/po_h[64:128]   (DVE reciprocal + mul, bf16)
  after each hf: out rows = ot^T @ Wo + bias (K=1 ones x bo matmul)

Projection groups are woven into the hf0 attention stream so the ScalarE
exp pipeline starts after just KT(p0)+QT(p0); kt/qt PSUM evacuations run
on ScalarE (idle then), V2 evacuations on DVE. Each nq-half's output
projection overlaps the other half's attention. Output is stored bf16 and
upcast on host.
"""

import numpy as np
from contextlib import ExitStack

import ml_dtypes
import concourse.bass as bass
import concourse.mybir as mybir
import concourse.tile as tile
from concourse import bacc
from concourse.bass_utils import run_bass_kernel_spmd

F32 = mybir.dt.float32
BF16 = mybir.dt.bfloat16
AF = mybir.ActivationFunctionType

NQ = 1024      # queries per core
NKV = 1024     # raw keys
NKV2 = 640     # compacted+padded keys
QD = 1024
CD = 768
H = 16
D = 64
INNER = 1024
SCALE = D ** -0.5
P = 128
NQC = NQ // P      # 8 nq chunks
NKC = NKV2 // P    # 5 key chunks
QDC = QD // P      # 8
CDC = CD // P      # 6
HP = H // 2        # 8 head pairs
NEG = -1e30


def _emit(tc, io):
    nc = tc.nc
    x_d, ctx_d, maskt_d, wq_d, wk_d, wv_d, wo_d, bo_d, out_d = io

    with ExitStack() as top:
        const = top.enter_context(tc.tile_pool(name="const", bufs=1))
        mb = const.tile([P, NKC], F32, tag="mb")  # mb[p, c] = maskb[c*128+p]

        big = top.enter_context(tc.tile_pool(name="big", bufs=1))
        ct = big.tile([P, CDC * NKV2], BF16, tag="ct")   # ctx^T: chunk k
        xt = big.tile([P, QDC * NQ], BF16, tag="xt")
        kt = big.tile([P, HP * NKV2], BF16, tag="kt")    # K^T: chunk p
        qt = big.tile([P, HP * NQ], BF16, tag="qt")
        # V2: block b = c*H + h at cols b*128: [V_h(c) (64) | ones (64)]
        vt = big.tile([P, NKC * H * P], BF16, tag="vt")
        ot = big.tile([P, QDC * NQ], BF16, tag="ot")     # O^T: chunk k

        vt4 = vt.rearrange("p (b n) -> p b n", n=P)

        wkp = top.enter_context(tc.tile_pool(name="wkp", bufs=CDC))
        wqp = top.enter_context(tc.tile_pool(name="wqp", bufs=QDC))
        wvp = top.enter_context(tc.tile_pool(name="wvp", bufs=CDC))
        wop = top.enter_context(tc.tile_pool(name="wop", bufs=QDC))
        wk = [wkp.tile([P, INNER], BF16, tag="wk", name=f"wk{k}")
              for k in range(CDC)]
        wq = [wqp.tile([P, INNER], BF16, tag="wq", name=f"wq{k}")
              for k in range(QDC)]
        wv = [wvp.tile([P, INNER], BF16, tag="wv", name=f"wv{k}")
              for k in range(CDC)]
        wo = [wop.tile([P, QD], BF16, tag="wo", name=f"wo{k}")
              for k in range(QDC)]

        # DMAs in first-use order; ct/wk interleaved so the KT accumulation
        # chain can start as soon as its k-th operands land.
        ct3 = ct.rearrange("p (k n) -> p k n", n=NKV2)
        xt3 = xt.rearrange("p (k n) -> p k n", n=NQ)
        for k in range(CDC):
            nc.sync.dma_start_transpose(
                ct3[:, k:k + 1, :], ctx_d[:, k * P:(k + 1) * P])
            nc.sync.dma_start(out=wk[k][:], in_=wk_d[k * P:(k + 1) * P, :])
        nc.sync.dma_start(out=mb[:], in_=maskt_d)
        for k in range(QDC):
            nc.sync.dma_start_transpose(
                xt3[:, k:k + 1, :], x_d[:, k * P:(k + 1) * P])
            nc.sync.dma_start(out=wq[k][:], in_=wq_d[k * P:(k + 1) * P, :])
        for k in range(CDC):
            nc.sync.dma_start(out=wv[k][:], in_=wv_d[k * P:(k + 1) * P, :])
        for k in range(QDC):
            nc.sync.dma_start(out=wo[k][:], in_=wo_d[k * P:(k + 1) * P, :])

        with tc.tile_pool(name="pj_ps", bufs=2, space="PSUM") as pj_ps, \
             tc.tile_pool(name="ps_o", bufs=2, space="PSUM") as ps_o, \
             tc.tile_pool(name="ps_s", bufs=2, space="PSUM") as ps_s, \
             tc.tile_pool(name="es", bufs=8) as es_pool, \
             tc.tile_pool(name="rt", bufs=2) as rt_pool, \
             tc.tile_pool(name="osb", bufs=3) as osb:

            def kt_group(p):
                # nkv2 = 640 split as N=512 + N=128
                for hf2, (n0, n1) in enumerate(((0, 512), (512, NKV2))):
                    ps = pj_ps.tile([P, 512], F32, tag="pj",
                                    name=f"pjk{p}_{hf2}")
                    psn = ps[:, 0:n1 - n0]
                    for k in range(CDC):
                        nc.tensor.matmul(
                            psn, wk[k][:, p * P:(p + 1) * P],
                            ct[:, k * NKV2 + n0: k * NKV2 + n1],
                            start=(k == 0), stop=(k == CDC - 1))
                    nc.scalar.copy(
                        kt[:, p * NKV2 + n0: p * NKV2 + n1], psn)

            def qt_group(p, hf):
                ps = pj_ps.tile([P, 512], F32, tag="pj", name=f"pjq{p}_{hf}")
                for k in range(QDC):
                    nc.tensor.matmul(
                        ps[:], wq[k][:, p * P:(p + 1) * P],
                        xt[:, k * NQ + hf * 512: k * NQ + (hf + 1) * 512],
                        start=(k == 0), stop=(k == QDC - 1))
                nc.scalar.copy(
                    qt[:, p * NQ + hf * 512: p * NQ + (hf + 1) * 512], ps[:])

            def v_group(c, hfi):
                ps = pj_ps.tile([P, 512], F32, tag="pj", name=f"pjv{c}_{hfi}")
                for k in range(CDC):
                    nc.tensor.matmul(
                        ps[:], ct[:, k * NKV2 + c * P: k * NKV2 + (c + 1) * P],
                        wv[k][:, hfi * 512:(hfi + 1) * 512],
                        start=(k == 0), stop=(k == CDC - 1))
                ps3 = ps.rearrange("p (j n) -> p j n", n=D)
                blk = vt4[:, c * H + 8 * hfi: c * H + 8 * hfi + 8, :]
                nc.vector.tensor_copy(blk[:, :, 0:D], ps3[:])
                nc.vector.memset(blk[:, :, D:P], 1.0)  # ones columns

            def attention(hf, p):
                po = [ps_o.tile([P, 512], F32, tag="po",
                                name=f"po{hf}_{p}_{hh}") for hh in range(2)]
                for c in range(NKC):
                    # S^T for both heads of the pair (row-tiled K=64)
                    ps = ps_s.tile([P, NQ], F32, tag="ss",
                                   name=f"ss{hf}_{p}_{c}")
                    for hh in range(2):
                        nc.tensor.matmul(
                            ps[:, hh * 512:(hh + 1) * 512],
                            kt[hh * D:(hh + 1) * D,
                               p * NKV2 + c * P: p * NKV2 + (c + 1) * P],
                            qt[hh * D:(hh + 1) * D,
                               p * NQ + hf * 512: p * NQ + (hf + 1) * 512],
                            start=True, stop=True,
                            tile_position=(hh * D, 0))
                    es = es_pool.tile([P, NQ], BF16, tag="es",
                                      name=f"es{hf}_{p}_{c}")
                    nc.scalar.activation(es[:], ps[:], AF.Exp,
                                         bias=mb[:, c:c + 1],
                                         scale=float(SCALE))
                    for hh in range(2):
                        h = 2 * p + hh
                        nc.tensor.matmul(
                            po[hh][:],
                            vt[:, (c * H + h) * P:(c * H + h + 1) * P],
                            es[:, hh * 512:(hh + 1) * 512],
                            start=(c == 0), stop=(c == NKC - 1),
                            skip_group_check=True)
                # normalize: rows 0-63 = O^T, rows 64-127 = denominator
                rt = rt_pool.tile([P, 512], F32, tag="rt", name=f"rt{hf}_{p}")
                with nc.allow_low_precision(reason="softmax reciprocal"):
                    nc.vector.reciprocal(rt[0:D, :], po[0][D:P, :])
                    nc.vector.reciprocal(rt[D:P, :], po[1][D:P, :])
                osl = ot[:, p * NQ + hf * 512: p * NQ + (hf + 1) * 512]
                nc.vector.tensor_mul(osl[0:D, :], po[0][0:D, :], rt[0:D, :])
                nc.vector.tensor_mul(osl[D:P, :], po[1][0:D, :], rt[D:P, :])

            def out_proj(m):
                for n2 in range(2):
                    # shares the pj ring (projection groups are done by now)
                    pso = pj_ps.tile([P, 512], F32, tag="pj",
                                     name=f"ops{m}_{n2}")
                    for k in range(QDC):
                        nc.tensor.matmul(
                            pso[:],
                            ot[:, k * NQ + m * P: k * NQ + (m + 1) * P],
                            wo[k][:, n2 * 512:(n2 + 1) * 512],
                            start=(k == 0), stop=(k == QDC - 1),
                            skip_group_check=True)
                    sb = osb.tile([P, 512], BF16, tag="osb",
                                  name=f"osb{m}_{n2}")
                    nc.vector.tensor_copy(sb[:], pso[:])
                    nc.sync.dma_start(
                        out=out_d[m * P:(m + 1) * P, n2 * 512:(n2 + 1) * 512],
                        in_=sb[:])

            # minimal prologue: just enough for attention(hf0, p0) to start,
            # remaining projection groups woven into the hf0 attention stream
            # to fill its ACT-bound PE slack. All emission is in dependency
            # order: every v/kt/qt group precedes its first reader.
            kt_group(0)
            qt_group(0, 0)
            for c in range(NKC):   # heads 0-7 V blocks: read from attention(0,0)
                v_group(c, 0)
            for p in range(HP):
                if p + 1 < HP:
                    kt_group(p + 1)
                    qt_group(p + 1, 0)
                if p < NKC:        # heads 8-15 V blocks: first read at p=4
                    v_group(p, 1)
                qt_group(p, 1)
                attention(0, p)
            for p in range(HP):
                attention(1, p)
                if p < 4:
                    out_proj(p)
            for m in range(4, NQC):
                out_proj(m)


_CACHED = {}


def _build(iters=1):
    if iters in _CACHED:
        return _CACHED[iters]
    nc = bacc.Bacc("TRN2", debug=False, target_bir_lowering=False)
    x = nc.dram_tensor("x", [NQ, QD], BF16, kind="ExternalInput").ap()
    ctx_t = nc.dram_tensor("ctx", [NKV2, CD], BF16, kind="ExternalInput").ap()
    maskt = nc.dram_tensor("maskt", [P, NKC], F32, kind="ExternalInput").ap()
    wq_d = nc.dram_tensor("wq", [QD, INNER], BF16, kind="ExternalInput").ap()
    wk_d = nc.dram_tensor("wk", [CD, INNER], BF16, kind="ExternalInput").ap()
    wv_d = nc.dram_tensor("wv", [CD, INNER], BF16, kind="ExternalInput").ap()
    wo_d = nc.dram_tensor("wo", [INNER, QD], BF16, kind="ExternalInput").ap()
    bo_d = nc.dram_tensor("bo", [QD], BF16, kind="ExternalInput").ap()
    out_d = nc.dram_tensor("out", [NQ, QD], BF16, kind="ExternalOutput").ap()
    io = (x, ctx_t, maskt, wq_d, wk_d, wv_d, wo_d, bo_d, out_d)
    with tile.TileContext(nc) as tc:
        for _ in range(iters):
            _emit(tc, io)
    nc.compile()
    _CACHED[iters] = nc
    return nc


def make_in_maps(x, context, mask, Wq, Wk, Wv, Wo, bo):
    bf = ml_dtypes.bfloat16
    x = np.asarray(x, dtype=np.float32).astype(bf)
    context = np.asarray(context, dtype=np.float32)
    mask = np.asarray(mask)
    # compact: gather each batch's unmasked context rows, pad to NKV2
    ctx_p = np.zeros((4, NKV2, CD), dtype=bf)
    maskb_p = np.full((4, NKV2), NEG, dtype=np.float32)
    for b in range(4):
        idx = np.nonzero(mask[b])[0]
        n = len(idx)
        assert n <= NKV2, f"batch {b} has {n} live keys > NKV2={NKV2}"
        ctx_p[b, :n] = context[b, idx].astype(bf)
        maskb_p[b, :n] = 0.0
    # maskt[p, c] = maskb_p[c*128 + p]  (host transpose -> contiguous DMA)
    maskt = np.ascontiguousarray(
        maskb_p.reshape(4, NKC, P).transpose(0, 2, 1))
    Wq = np.ascontiguousarray(np.asarray(Wq, dtype=np.float32).astype(bf))
    Wk = np.ascontiguousarray(np.asarray(Wk, dtype=np.float32).astype(bf))
    Wv = np.ascontiguousarray(np.asarray(Wv, dtype=np.float32).astype(bf))
    Wo = np.ascontiguousarray(np.asarray(Wo, dtype=np.float32).astype(bf))
    bo = np.ascontiguousarray(np.asarray(bo, dtype=np.float32).astype(bf))
    in_maps = []
    for b in range(4):
        for qh in range(2):
            in_maps.append({
                "x": np.ascontiguousarray(x[b, qh * NQ:(qh + 1) * NQ, :]),
                "ctx": ctx_p[b],
                "maskt": maskt[b],
                "wq": Wq, "wk": Wk, "wv": Wv, "wo": Wo, "bo": bo,
            })
    return in_maps


def run_sharded(x, context, mask, Wq, Wk, Wv, Wo, bo, trace=False, **kw):
    nc = _build()
    in_maps = make_in_maps(x, context, mask, Wq, Wk, Wv, Wo, bo)
    res = run_bass_kernel_spmd(nc, in_maps, list(range(8)), trace=trace, **kw)
    bo_f = np.asarray(bo, dtype=np.float32)
    out = np.empty((4, 2 * NQ, QD), dtype=np.float32)
    for i in range(8):
        b, qh = divmod(i, 2)
        out[b, qh * NQ:(qh + 1) * NQ, :] = \
            np.asarray(res.results[i]["out"]).astype(np.float32) + bo_f
    return out, res


def kernel(x, context, mask, Wq, Wk, Wv, Wo, bo):
    out, _ = run_sharded(x, context, mask, Wq, Wk, Wv, Wo, bo, trace=False)
    return out


# revision 30
# speedup vs baseline: 3.6219x; 3.6219x over previous
"""Trainium2 Bass kernel for cross-attention (b=4, nq=2048, nkv=1024,
qdim=1024, cdim=768, heads=16, dim_head=64).

Sharding: 8 cores = batch(4) x nq-half(2). Each core computes a disjoint
[1024, 1024] slice of the output; no collectives needed.

Key compaction: the boolean key-mask drops ~half the keys, and masked keys
contribute exactly zero to softmax(QK^T)V, so the host gathers each batch's
unmasked context rows and pads to NKV2=640 (seed-0 batches have <=536 live
keys; 640 = mean + 8 sigma of Binomial(1024, 1/2), and padding rows are
forced to zero attention via the exp bias). All key-side work (K/V
projections, S, exp, O) shrinks by 640/1024.

Per-core algorithm (bf16 operands, f32 PSUM accumulation):
  CT = ctx_packed^T, XT = x^T       (xbar DMA-transposed loads, no PE time)
  KT = Wk^T @ CT   [inner, nkv2]    (inner chunk p holds heads 2p, 2p+1)
  QT = Wq^T @ XT   [inner, nq]
  V2 = CT^T @ Wv   [nkv2, inner]    stored as per-(chunk, head) blocks
                                    [V_h (64 cols) | ones (64 cols)]
  per hf (nq 512-half), per head-pair p, per key-chunk c:
    S^T = K_h @ Q_h^T               (row-tiled K=64: 2 heads concurrent)
    ES  = exp(SCALE*S^T + mbias_j)  (ScalarE; mbias = 0 live / -1e30 pad)
    po_h += [V_h | ones]^T @ ES_h   (M=128: rows 0-63 = O^T, 64-127 = the
                                     softmax denominator, broadcast x64)
  ot_h = po_h[0:64] * 1/po_h[64:128]   (DVE reciprocal + mul, bf16)
  after each hf: out rows = ot^T @ Wo + bias (K=1 ones x bo matmul)

Projection groups are woven into the hf0 attention stream so the ScalarE
exp pipeline starts after just KT(p0)+QT(p0); kt/qt PSUM evacuations run
on ScalarE (idle then), V2 evacuations on DVE. Each nq-half's output
projection overlaps the other half's attention. Output is stored bf16 and
upcast on host.
"""

import numpy as np
from contextlib import ExitStack

import ml_dtypes
import concourse.bass as bass
import concourse.mybir as mybir
import concourse.tile as tile
from concourse import bacc
from concourse.bass_utils import run_bass_kernel_spmd

F32 = mybir.dt.float32
BF16 = mybir.dt.bfloat16
AF = mybir.ActivationFunctionType

NQ = 1024      # queries per core
NKV = 1024     # raw keys
NKV2 = 640     # compacted+padded keys
QD = 1024
CD = 768
H = 16
D = 64
INNER = 1024
SCALE = D ** -0.5
P = 128
NQC = NQ // P      # 8 nq chunks
NKC = NKV2 // P    # 5 key chunks
QDC = QD // P      # 8
CDC = CD // P      # 6
HP = H // 2        # 8 head pairs
NEG = -1e30


def _emit(tc, io):
    nc = tc.nc
    x_d, ctx_d, maskt_d, wq_d, wk_d, wv_d, wo_d, bo_d, out_d = io

    with ExitStack() as top:
        const = top.enter_context(tc.tile_pool(name="const", bufs=1))
        mb = const.tile([P, NKC], F32, tag="mb")  # mb[p, c] = maskb[c*128+p]

        big = top.enter_context(tc.tile_pool(name="big", bufs=1))
        ct = big.tile([P, CDC * NKV2], BF16, tag="ct")   # ctx^T: chunk k
        xt = big.tile([P, QDC * NQ], BF16, tag="xt")
        kt = big.tile([P, HP * NKV2], BF16, tag="kt")    # K^T: chunk p
        qt = big.tile([P, HP * NQ], BF16, tag="qt")
        # V2: block b = c*H + h at cols b*128: [V_h(c) (64) | ones (64)]
        vt = big.tile([P, NKC * H * P], BF16, tag="vt")
        ot = big.tile([P, QDC * NQ], BF16, tag="ot")     # O^T: chunk k

        vt4 = vt.rearrange("p (b n) -> p b n", n=P)

        wkp = top.enter_context(tc.tile_pool(name="wkp", bufs=CDC))
        wqp = top.enter_context(tc.tile_pool(name="wqp", bufs=QDC))
        wvp = top.enter_context(tc.tile_pool(name="wvp", bufs=CDC))
        wop = top.enter_context(tc.tile_pool(name="wop", bufs=QDC))
        wk = [wkp.tile([P, INNER], BF16, tag="wk", name=f"wk{k}")
              for k in range(CDC)]
        wq = [wqp.tile([P, INNER], BF16, tag="wq", name=f"wq{k}")
              for k in range(QDC)]
        wv = [wvp.tile([P, INNER], BF16, tag="wv", name=f"wv{k}")
              for k in range(CDC)]
        wo = [wop.tile([P, QD], BF16, tag="wo", name=f"wo{k}")
              for k in range(QDC)]

        # DMAs in first-use order; ct/wk interleaved so the KT accumulation
        # chain can start as soon as its k-th operands land.
        ct3 = ct.rearrange("p (k n) -> p k n", n=NKV2)
        xt3 = xt.rearrange("p (k n) -> p k n", n=NQ)
        for k in range(CDC):
            nc.sync.dma_start_transpose(
                ct3[:, k:k + 1, :], ctx_d[:, k * P:(k + 1) * P])
            nc.sync.dma_start(out=wk[k][:], in_=wk_d[k * P:(k + 1) * P, :])
        nc.sync.dma_start(out=mb[:], in_=maskt_d)
        for k in range(QDC):
            nc.sync.dma_start_transpose(
                xt3[:, k:k + 1, :], x_d[:, k * P:(k + 1) * P])
            nc.sync.dma_start(out=wq[k][:], in_=wq_d[k * P:(k + 1) * P, :])
        for k in range(CDC):
            nc.sync.dma_start(out=wv[k][:], in_=wv_d[k * P:(k + 1) * P, :])
        for k in range(QDC):
            nc.sync.dma_start(out=wo[k][:], in_=wo_d[k * P:(k + 1) * P, :])

        with tc.tile_pool(name="pj_ps", bufs=2, space="PSUM") as pj_ps, \
             tc.tile_pool(name="ps_o", bufs=2, space="PSUM") as ps_o, \
             tc.tile_pool(name="ps_s", bufs=2, space="PSUM") as ps_s, \
             tc.tile_pool(name="es", bufs=8) as es_pool, \
             tc.tile_pool(name="rt", bufs=3) as rt_pool, \
             tc.tile_pool(name="osb", bufs=4) as osb:

            def kt_group(p):
                # nkv2 = 640 split as N=512 + N=128
                for hf2, (n0, n1) in enumerate(((0, 512), (512, NKV2))):
                    ps = pj_ps.tile([P, 512], F32, tag="pj",
                                    name=f"pjk{p}_{hf2}")
                    psn = ps[:, 0:n1 - n0]
                    for k in range(CDC):
                        nc.tensor.matmul(
                            psn, wk[k][:, p * P:(p + 1) * P],
                            ct[:, k * NKV2 + n0: k * NKV2 + n1],
                            start=(k == 0), stop=(k == CDC - 1))
                    nc.scalar.copy(
                        kt[:, p * NKV2 + n0: p * NKV2 + n1], psn)

            def qt_group(p, hf):
                ps = pj_ps.tile([P, 512], F32, tag="pj", name=f"pjq{p}_{hf}")
                for k in range(QDC):
                    nc.tensor.matmul(
                        ps[:], wq[k][:, p * P:(p + 1) * P],
                        xt[:, k * NQ + hf * 512: k * NQ + (hf + 1) * 512],
                        start=(k == 0), stop=(k == QDC - 1))
                nc.scalar.copy(
                    qt[:, p * NQ + hf * 512: p * NQ + (hf + 1) * 512], ps[:])

            def v_group(c, hfi):
                ps = pj_ps.tile([P, 512], F32, tag="pj", name=f"pjv{c}_{hfi}")
                for k in range(CDC):
                    nc.tensor.matmul(
                        ps[:], ct[:, k * NKV2 + c * P: k * NKV2 + (c + 1) * P],
                        wv[k][:, hfi * 512:(hfi + 1) * 512],
                        start=(k == 0), stop=(k == CDC - 1))
                ps3 = ps.rearrange("p (j n) -> p j n", n=D)
                blk = vt4[:, c * H + 8 * hfi: c * H + 8 * hfi + 8, :]
                nc.vector.tensor_copy(blk[:, :, 0:D], ps3[:])
                nc.vector.memset(blk[:, :, D:P], 1.0)  # ones columns

            def attention(hf, p):
                po = [ps_o.tile([P, 512], F32, tag="po",
                                name=f"po{hf}_{p}_{hh}") for hh in range(2)]
                for c in range(NKC):
                    # S^T for both heads of the pair (row-tiled K=64)
                    ps = ps_s.tile([P, NQ], F32, tag="ss",
                                   name=f"ss{hf}_{p}_{c}")
                    for hh in range(2):
                        nc.tensor.matmul(
                            ps[:, hh * 512:(hh + 1) * 512],
                            kt[hh * D:(hh + 1) * D,
                               p * NKV2 + c * P: p * NKV2 + (c + 1) * P],
                            qt[hh * D:(hh + 1) * D,
                               p * NQ + hf * 512: p * NQ + (hf + 1) * 512],
                            start=True, stop=True,
                            tile_position=(hh * D, 0))
                    es = es_pool.tile([P, NQ], BF16, tag="es",
                                      name=f"es{hf}_{p}_{c}")
                    nc.scalar.activation(es[:], ps[:], AF.Exp,
                                         bias=mb[:, c:c + 1],
                                         scale=float(SCALE))
                    for hh in range(2):
                        h = 2 * p + hh
                        nc.tensor.matmul(
                            po[hh][:],
                            vt[:, (c * H + h) * P:(c * H + h + 1) * P],
                            es[:, hh * 512:(hh + 1) * 512],
                            start=(c == 0), stop=(c == NKC - 1),
                            skip_group_check=True)
                # normalize: rows 0-63 = O^T, rows 64-127 = denominator
                rt = rt_pool.tile([P, 512], F32, tag="rt", name=f"rt{hf}_{p}")
                with nc.allow_low_precision(reason="softmax reciprocal"):
                    nc.vector.reciprocal(rt[0:D, :], po[0][D:P, :])
                    nc.vector.reciprocal(rt[D:P, :], po[1][D:P, :])
                osl = ot[:, p * NQ + hf * 512: p * NQ + (hf + 1) * 512]
                nc.vector.tensor_mul(osl[0:D, :], po[0][0:D, :], rt[0:D, :])
                nc.vector.tensor_mul(osl[D:P, :], po[1][0:D, :], rt[D:P, :])

            def out_proj(m):
                for n2 in range(2):
                    # shares the pj ring (projection groups are done by now)
                    pso = pj_ps.tile([P, 512], F32, tag="pj",
                                     name=f"ops{m}_{n2}")
                    for k in range(QDC):
                        nc.tensor.matmul(
                            pso[:],
                            ot[:, k * NQ + m * P: k * NQ + (m + 1) * P],
                            wo[k][:, n2 * 512:(n2 + 1) * 512],
                            start=(k == 0), stop=(k == QDC - 1),
                            skip_group_check=True)
                    sb = osb.tile([P, 512], BF16, tag="osb",
                                  name=f"osb{m}_{n2}")
                    nc.vector.tensor_copy(sb[:], pso[:])
                    nc.sync.dma_start(
                        out=out_d[m * P:(m + 1) * P, n2 * 512:(n2 + 1) * 512],
                        in_=sb[:])

            # minimal prologue: just enough for attention(hf0, p0) to start,
            # remaining projection groups woven into the hf0 attention stream
            # to fill its ACT-bound PE slack. All emission is in dependency
            # order: every v/kt/qt group precedes its first reader.
            kt_group(0)
            qt_group(0, 0)
            for c in range(NKC):   # heads 0-7 V blocks: read from attention(0,0)
                v_group(c, 0)
            for p in range(HP):
                if p + 1 < HP:
                    kt_group(p + 1)
                    qt_group(p + 1, 0)
                if p < NKC:        # heads 8-15 V blocks: first read at p=4
                    v_group(p, 1)
                qt_group(p, 1)
                attention(0, p)
            for p in range(HP):
                attention(1, p)
                if p < 4:
                    out_proj(p)
            for m in range(4, NQC):
                out_proj(m)


_CACHED = {}


def _build(iters=1):
    if iters in _CACHED:
        return _CACHED[iters]
    nc = bacc.Bacc("TRN2", debug=False, target_bir_lowering=False)
    x = nc.dram_tensor("x", [NQ, QD], BF16, kind="ExternalInput").ap()
    ctx_t = nc.dram_tensor("ctx", [NKV2, CD], BF16, kind="ExternalInput").ap()
    maskt = nc.dram_tensor("maskt", [P, NKC], F32, kind="ExternalInput").ap()
    wq_d = nc.dram_tensor("wq", [QD, INNER], BF16, kind="ExternalInput").ap()
    wk_d = nc.dram_tensor("wk", [CD, INNER], BF16, kind="ExternalInput").ap()
    wv_d = nc.dram_tensor("wv", [CD, INNER], BF16, kind="ExternalInput").ap()
    wo_d = nc.dram_tensor("wo", [INNER, QD], BF16, kind="ExternalInput").ap()
    bo_d = nc.dram_tensor("bo", [QD], BF16, kind="ExternalInput").ap()
    out_d = nc.dram_tensor("out", [NQ, QD], BF16, kind="ExternalOutput").ap()
    io = (x, ctx_t, maskt, wq_d, wk_d, wv_d, wo_d, bo_d, out_d)
    with tile.TileContext(nc) as tc:
        for _ in range(iters):
            _emit(tc, io)
    nc.compile()
    _CACHED[iters] = nc
    return nc


def make_in_maps(x, context, mask, Wq, Wk, Wv, Wo, bo):
    bf = ml_dtypes.bfloat16
    x = np.asarray(x, dtype=np.float32).astype(bf)
    context = np.asarray(context, dtype=np.float32)
    mask = np.asarray(mask)
    # compact: gather each batch's unmasked context rows, pad to NKV2
    ctx_p = np.zeros((4, NKV2, CD), dtype=bf)
    maskb_p = np.full((4, NKV2), NEG, dtype=np.float32)
    for b in range(4):
        idx = np.nonzero(mask[b])[0]
        n = len(idx)
        assert n <= NKV2, f"batch {b} has {n} live keys > NKV2={NKV2}"
        ctx_p[b, :n] = context[b, idx].astype(bf)
        maskb_p[b, :n] = 0.0
    # maskt[p, c] = maskb_p[c*128 + p]  (host transpose -> contiguous DMA)
    maskt = np.ascontiguousarray(
        maskb_p.reshape(4, NKC, P).transpose(0, 2, 1))
    Wq = np.ascontiguousarray(np.asarray(Wq, dtype=np.float32).astype(bf))
    Wk = np.ascontiguousarray(np.asarray(Wk, dtype=np.float32).astype(bf))
    Wv = np.ascontiguousarray(np.asarray(Wv, dtype=np.float32).astype(bf))
    Wo = np.ascontiguousarray(np.asarray(Wo, dtype=np.float32).astype(bf))
    bo = np.ascontiguousarray(np.asarray(bo, dtype=np.float32).astype(bf))
    in_maps = []
    for b in range(4):
        for qh in range(2):
            in_maps.append({
                "x": np.ascontiguousarray(x[b, qh * NQ:(qh + 1) * NQ, :]),
                "ctx": ctx_p[b],
                "maskt": maskt[b],
                "wq": Wq, "wk": Wk, "wv": Wv, "wo": Wo, "bo": bo,
            })
    return in_maps


def run_sharded(x, context, mask, Wq, Wk, Wv, Wo, bo, trace=False, **kw):
    nc = _build()
    in_maps = make_in_maps(x, context, mask, Wq, Wk, Wv, Wo, bo)
    res = run_bass_kernel_spmd(nc, in_maps, list(range(8)), trace=trace, **kw)
    bo_f = np.asarray(bo, dtype=np.float32)
    out = np.empty((4, 2 * NQ, QD), dtype=np.float32)
    for i in range(8):
        b, qh = divmod(i, 2)
        out[b, qh * NQ:(qh + 1) * NQ, :] = \
            np.asarray(res.results[i]["out"]).astype(np.float32) + bo_f
    return out, res


def kernel(x, context, mask, Wq, Wk, Wv, Wo, bo):
    out, _ = run_sharded(x, context, mask, Wq, Wk, Wv, Wo, bo, trace=False)
    return out
